# revision 4
# baseline (speedup 1.0000x reference)
"""ChebNet (K=3, 3 conv layers + MLP head) on 8 Trainium2 NeuronCores.

Strategy: destination-node sharding. Node features h stay replicated in each
core's HBM; each core owns 1/8 of the destination nodes and all edges into
them. A propagate is: dma_gather of h[src] rows (256B each), a per-edge norm
scale on DVE, and a segment-sum via PE matmuls (gathered chunk stationary,
data-dependent one-hot built on DVE as the moving operand), accumulating
feature-major results in PSUM. Shard outputs are exchanged with AllGather
collectives (the graph is random, so halo == everything; full replication of
h is the right call). The small 64x64 weights are replicated; pooling is a
partial segment-sum per shard + one AllGather + local combine.
"""

import numpy as np

N = 50000
E = 800000
F = 64
H = 64
C = 10
G = 512
K = 3
NCORES = 8
NP = 50176          # padded node count: 8 * 6272
SHARD = NP // NCORES  # 6272 = 49 * 128
NBLK = SHARD // 128   # 49 dst blocks of 128 nodes per core
GRP = 4               # blocks per gather group
NGRP = (NBLK + GRP - 1) // GRP  # 13
LO_ROWS = 32768       # int16 gather index limit
HI_ROWS = NP - LO_ROWS
OHB = 8               # one-hot build batch (chunks per DVE op)

_cache = {}


def _prep(x, edge_index, edge_weight, batch):
    """All host-side graph structure preprocessing (numpy)."""
    src = np.asarray(edge_index[0], np.int64)
    dst = np.asarray(edge_index[1], np.int64)
    ew = np.asarray(edge_weight, np.float64)
    w0 = np.where(src == dst, 0.0, ew)
    deg = np.bincount(src, weights=w0, minlength=NP).astype(np.float64)
    dis = np.where(deg > 0, 1.0 / np.sqrt(np.where(deg > 0, deg, 1.0)), 0.0)
    norm = (-dis[src] * w0 * dis[dst]).astype(np.float32)

    core = dst // SHARD
    blk = (dst % SHARD) // 128
    sec = (src >= LO_ROWS).astype(np.int64)  # 0 = lo, 1 = hi

    # order edges by (core, blk, sec) and count
    counts = np.zeros((NCORES, NBLK, 2), np.int64)
    np.add.at(counts, (core, blk, sec), 1)
    budgets = np.maximum(1, np.ceil(counts.max(axis=0) / 128).astype(np.int64))

    # chunk schedule (identical on every core): per group g: lo chunks of its
    # blocks, then hi chunks.
    sched = []  # list of (g, s, b) per chunk, in program order
    ginfo = []  # per (g, s): (chunk_start, nchunks)
    for g in range(NGRP):
        blks = range(g * GRP, min((g + 1) * GRP, NBLK))
        for s in (0, 1):
            start = len(sched)
            for b in blks:
                for _ in range(budgets[b, s]):
                    sched.append((g, s, b))
            ginfo.append((g, s, start, len(sched) - start))
    nch = len(sched)

    # per-core streams
    order = np.lexsort((src, sec, blk, core))  # stable order by core,blk,sec
    so, do, no, co, bo, seco = (a[order] for a in (src, dst, norm, core, blk, sec))
    idx_arr = np.zeros((NCORES, nch * 128), np.int16)
    ld_arr = np.zeros((NCORES, 128, nch), np.float32)
    nm_arr = np.zeros((NCORES, 128, nch), np.float32)

    # chunk offsets per (b, s): start chunk of block b in section s
    chunk_of = {}
    pos = 0
    for g, s, start, n in ginfo:
        blks = list(range(g * GRP, min((g + 1) * GRP, NBLK)))
        cstart = start
        for b in blks:
            chunk_of[(b, s)] = cstart
            cstart += budgets[b, s]

    eptr = np.searchsorted(co, np.arange(NCORES + 1))
    for c in range(NCORES):
        s0, s1 = eptr[c], eptr[c + 1]
        bsec = bo[s0:s1] * 2 + seco[s0:s1]
        bs_ptr = np.searchsorted(bsec, np.arange(2 * NBLK + 1))
        for b in range(NBLK):
            for s in (0, 1):
                lo_, hi_ = bs_ptr[2 * b + s], bs_ptr[2 * b + s + 1]
                cnt = hi_ - lo_
                ck0 = chunk_of[(b, s)]
                sl = slice(s0 + lo_, s0 + hi_)
                e_idx = (so[sl] - (LO_ROWS if s else 0)).astype(np.int16)
                e_ld = (do[sl] % 128).astype(np.float32)
                e_nm = no[sl]
                flat0 = ck0 * 128
                idx_arr[c, flat0:flat0 + cnt] = e_idx
                pp = np.arange(cnt)
                ld_arr[c, pp % 128, ck0 + pp // 128] = e_ld
                nm_arr[c, pp % 128, ck0 + pp // 128] = e_nm

    # wrap idx per gather instruction: [(16, ni//16) -> tile 8x]
    idx_w = np.zeros((NCORES, 128, nch * 8), np.int16)
    for g, s, start, n in ginfo:
        ni = n * 128
        c0 = start * 8
        for c in range(NCORES):
            seg = idx_arr[c, start * 128:start * 128 + ni]
            idx_w[c, :, c0:c0 + ni // 16] = np.tile(
                seg.reshape(ni // 16, 16).T, (8, 1))

    # pooling: graph slots
    batch = np.asarray(batch, np.int64)
    batch_p = np.concatenate([batch, np.full(NP - N, -1, np.int64)])
    gbase = np.zeros(NCORES, np.int64)
    gspan = np.zeros(NCORES, np.int64)
    gslot = np.zeros((NCORES, 128, NBLK), np.float32)
    for c in range(NCORES):
        bseg = batch_p[c * SHARD:(c + 1) * SHARD]
        real = bseg >= 0
        gbase[c] = bseg[real].min()
        gspan[c] = bseg[real].max() - gbase[c] + 1
        slot = np.where(real, bseg - gbase[c], 127).astype(np.float32)
        gslot[c] = slot.reshape(NBLK, 128).T
    assert gspan.max() <= 127

    return dict(budgets=budgets, ginfo=ginfo, sched=sched, nch=nch,
                chunk_of=chunk_of, idx_w=idx_w, ld=ld_arr, nm=nm_arr,
                gslot=gslot, gbase=gbase, gspan=gspan)


def _build(meta):
    import concourse.bacc as bacc
    import concourse.mybir as mybir
    import concourse.tile as tile

    fp32 = mybir.dt.float32
    Alu = mybir.AluOpType
    Act = mybir.ActivationFunctionType
    nch = meta["nch"]
    ginfo = meta["ginfo"]
    budgets = meta["budgets"]
    chunk_of = meta["chunk_of"]
    gbase, gspan = meta["gbase"], meta["gspan"]

    nc = bacc.Bacc("TRN2", target_bir_lowering=False, num_devices=NCORES,
                   num_swdge_queues=4)

    x_dram = nc.dram_tensor("x_full", [NP, F], fp32, kind="ExternalInput")
    xT_in = nc.dram_tensor("xT_shard", [F, SHARD], fp32, kind="ExternalInput")
    idx_in = nc.dram_tensor("idxw", [128, nch * 8], mybir.dt.int16,
                            kind="ExternalInput")
    ld_in = nc.dram_tensor("ld", [128, nch], fp32, kind="ExternalInput")
    nm_in = nc.dram_tensor("nm", [128, nch], fp32, kind="ExternalInput")
    gs_in = nc.dram_tensor("gslot", [128, NBLK], fp32, kind="ExternalInput")
    iota_in = nc.dram_tensor("iota8", [128, OHB * 128], fp32,
                             kind="ExternalInput")
    ident_in = nc.dram_tensor("ident", [128, 128], fp32, kind="ExternalInput")
    w_in = nc.dram_tensor("Wall", [F, 3 * K * H], fp32, kind="ExternalInput")
    b_in = nc.dram_tensor("ball", [H, 3], fp32, kind="ExternalInput")
    l1w_in = nc.dram_tensor("l1w", [H, H], fp32, kind="ExternalInput")
    l1b_in = nc.dram_tensor("l1b", [H, 1], fp32, kind="ExternalInput")
    l2aug_in = nc.dram_tensor("l2aug", [H + 1, C], fp32, kind="ExternalInput")
    y_out = nc.dram_tensor("y", [G, C], fp32, kind="ExternalOutput")

    with tile.TileContext(nc) as tc:
        with (
            tc.tile_pool(name="persist", bufs=1) as pp,
            tc.tile_pool(name="gpool", bufs=2) as gp_,
            tc.tile_pool(name="ohpool", bufs=3) as ohp,
            tc.tile_pool(name="small", bufs=2) as sp,
            tc.tile_pool(name="psA", bufs=2, space="PSUM") as psA,
            tc.tile_pool(name="psB", bufs=3, space="PSUM") as psB,
            tc.tile_pool(name="dram", bufs=1, space="DRAM") as dp,
        ):
            # ---- static loads ----
            idx_sb = pp.tile([128, nch * 8], mybir.dt.int16, tag="idx")
            nc.sync.dma_start(idx_sb[:], idx_in[:])
            ld_sb = pp.tile([128, nch], fp32, tag="ld")
            nc.sync.dma_start(ld_sb[:], ld_in[:])
            nm_sb = pp.tile([128, nch], fp32, tag="nm")
            nc.sync.dma_start(nm_sb[:], nm_in[:])
            gs_sb = pp.tile([128, NBLK], fp32, tag="gs")
            nc.sync.dma_start(gs_sb[:], gs_in[:])
            iota = pp.tile([128, OHB * 128], fp32, tag="iota")
            nc.sync.dma_start(iota[:], iota_in[:])
            ident = pp.tile([128, 128], fp32, tag="ident")
            nc.sync.dma_start(ident[:], ident_in[:])
            w_sb = pp.tile([F, 3 * K * H], fp32, tag="w")
            nc.sync.dma_start(w_sb[:], w_in[:])
            b_sb = pp.tile([H, 3], fp32, tag="b")
            nc.sync.dma_start(b_sb[:], b_in[:])
            l1w = pp.tile([H, H], fp32, tag="l1w")
            nc.sync.dma_start(l1w[:], l1w_in[:])
            l1b = pp.tile([H, 1], fp32, tag="l1b")
            nc.sync.dma_start(l1b[:], l1b_in[:])
            l2aug = pp.tile([H + 1, C], fp32, tag="l2aug")
            nc.sync.dma_start(l2aug[:], l2aug_in[:])

            # feature-major activation tiles [64, SHARD]
            tx0 = pp.tile([F, SHARD], fp32, tag="tx0")
            tx1 = pp.tile([F, SHARD], fp32, tag="tx1")
            tx2 = pp.tile([F, SHARD], fp32, tag="tx2")
            stage = pp.tile([128, NBLK * F], fp32, tag="stage")
            nc.sync.dma_start(tx0[:], xT_in[:])

            # DRAM comm buffers
            agin = [dp.tile([SHARD, F], fp32, tag=f"agin{i}", name=f"agin{i}") for i in range(5)]
            agout = [dp.tile([NP, F], fp32, tag=f"agout{i}", name=f"agout{i}") for i in range(5)]

            def propagate(src_dram, zT):
                """zT[:, :] = feature-major propagate of src_dram rows."""
                for g in range(NGRP):
                    blks = list(range(g * GRP, min((g + 1) * GRP, NBLK)))
                    ncols = len(blks) * 128
                    ps = psA.tile([F, GRP * 128], fp32, tag="big")
                    gt = {}
                    for s in (0, 1):
                        _, _, start, n = ginfo[g * 2 + s]
                        if n == 0:
                            continue
                        gtile = gp_.tile([128, n, F], fp32, tag=f"g{s}")
                        base = src_dram[LO_ROWS:NP, :] if s else \
                            src_dram[0:LO_ROWS, :]
                        nc.gpsimd.dma_gather(
                            out_ap=gtile[:],
                            in_ap=base,
                            idxs_ap=idx_sb[:, start * 8:(start + n) * 8],
                            num_idxs=n * 128,
                            num_idxs_reg=n * 128,
                            elem_size=F,
                            queue_num=(g * 2 + s) % 4,
                            single_packet=False,
                        )
                        # per-edge norm scale (broadcast norm along features)
                        nc.vector.tensor_tensor(
                            out=gtile[:], in0=gtile[:],
                            in1=nm_sb[:, start:start + n].unsqueeze(2)
                            .broadcast_to([128, n, F]),
                            op=Alu.mult)
                        gt[s] = (gtile, start, n)
                    # one-hot builds (batched) + matmuls
                    for s in (0, 1):
                        if s not in gt:
                            continue
                        gtile, start, n = gt[s]
                        for c0 in range(0, n, OHB):
                            nb = min(OHB, n - c0)
                            oh = ohp.tile([128, OHB, 128], fp32, tag="oh")
                            nc.vector.tensor_tensor(
                                out=oh[:, 0:nb, :],
                                in0=iota[:].rearrange(
                                    "p (c f) -> p c f", f=128)[:, 0:nb, :],
                                in1=ld_sb[:, start + c0:start + c0 + nb]
                                .unsqueeze(2).broadcast_to([128, nb, 128]),
                                op=Alu.is_equal)
                            for cc in range(nb):
                                ch = start + c0 + cc
                                _, ss, bb = meta["sched"][ch]
                                bi = bb - blks[0]
                                first = (ch == chunk_of[(bb, 0)])
                                last = (ch == chunk_of[(bb, 1)] +
                                        budgets[bb, 1] - 1)
                                nc.tensor.matmul(
                                    ps[:, bi * 128:(bi + 1) * 128],
                                    gtile[:, c0 + cc, :],
                                    oh[:, cc, :],
                                    start=first, stop=last)
                    nc.scalar.activation(
                        zT[:, g * GRP * 128:g * GRP * 128 + ncols],
                        ps[:, 0:ncols], Act.Copy)

            def transpose_back(zT, out_stage):
                """[64, SHARD] feature-major -> node-major [128, NBLK, 64]."""
                for t in range(NBLK):
                    tp = psB.tile([128, F], fp32, tag="tp")
                    nc.tensor.transpose(
                        tp[:], zT[:, t * 128:(t + 1) * 128], ident[0:F, 0:F])
                    nc.scalar.activation(
                        out_stage[:, t * F:(t + 1) * F], tp[:], Act.Copy)

            def exchange(zT, idx):
                transpose_back(zT, stage)
                nc.sync.dma_start(
                    agin[idx][:].rearrange("(t p) f -> p t f", p=128),
                    stage[:].rearrange("p (t f) -> p t f", f=F))
                nc.gpsimd.collective_compute(
                    "AllGather", mybir.AluOpType.bypass,
                    replica_groups=[list(range(NCORES))],
                    ins=[agin[idx].opt()],
                    outs=[agout[idx].opt()])
                return agout[idx]

            # ---- 3 conv layers ----
            slots = [(tx0, tx1, tx2, tx2), (tx2, tx1, tx0, tx0),
                     (tx0, tx1, tx2, tx2)]
            src = x_dram
            agi = 0
            for L in range(3):
                t0, t1, t2, ho = slots[L]
                propagate(src, t1)
                t1full = exchange(t1, agi)
                agi += 1
                propagate(t1full, t2)
                # t2 = 2*t2 - t0
                nc.vector.tensor_scalar(
                    out=t2[:], in0=t2[:], scalar1=2.0, scalar2=None,
                    op0=Alu.mult)
                nc.vector.tensor_tensor(
                    out=t2[:], in0=t2[:], in1=t0[:], op=Alu.subtract)
                # combo: ho = relu(W0^T t0 + W1^T t1 + W2^T t2 + b)
                for tt in range(NBLK * 128 // 512):
                    cs = tt * 512
                    cp = psA.tile([F, 512], fp32, tag="big")
                    for k, tk in enumerate((t0, t1, t2)):
                        wk = w_sb[:, (L * K + k) * H:(L * K + k + 1) * H]
                        nc.tensor.matmul(
                            cp[:], wk, tk[:, cs:cs + 512],
                            start=(k == 0), stop=(k == 2))
                    nc.scalar.activation(
                        ho[:, cs:cs + 512], cp[:], Act.Relu,
                        bias=b_sb[:, L:L + 1])
                if L < 2:
                    src = exchange(ho, agi)
                    agi += 1

            # ---- pooling (h3 = hout of conv3 = slots[2][3]) ----
            h3 = slots[2][3]
            h3aug = pp.tile([128, NBLK, F + 1], fp32, tag="h3aug")
            nc.vector.memset(h3aug[:, :, F:F + 1], 1.0)
            for t in range(NBLK):
                tp = psB.tile([128, F], fp32, tag="tp")
                nc.tensor.transpose(
                    tp[:], h3[:, t * 128:(t + 1) * 128], ident[0:F, 0:F])
                nc.scalar.activation(h3aug[:, t, 0:F], tp[:], Act.Copy)
            plp = psA.tile([F + 1, 512], fp32, tag="big")
            for t in range(NBLK):
                goh = ohp.tile([128, 128], fp32, tag="goh")
                nc.vector.tensor_scalar(
                    out=goh[:], in0=iota[:, 0:128],
                    scalar1=gs_sb[:, t:t + 1], scalar2=None, op0=Alu.is_equal)
                nc.tensor.matmul(plp[:, 0:128], h3aug[:, t, :], goh[:],
                                 start=(t == 0), stop=(t == NBLK - 1))
            ppart = sp.tile([F + 1, 128], fp32, tag="ppart")
            nc.scalar.activation(ppart[:], plp[:, 0:128], Act.Copy)

            agp_in = dp.tile([F + 1, 128], fp32, tag="agpin")
            agp_out = dp.tile([(F + 1) * NCORES, 128], fp32, tag="agpout")
            nc.sync.dma_start(agp_in[:], ppart[:])
            nc.gpsimd.collective_compute(
                "AllGather", mybir.AluOpType.bypass,
                replica_groups=[list(range(NCORES))],
                ins=[agp_in.opt()], outs=[agp_out.opt()])

            # combine partial pools -> gpool [65, G]
            gpo = pp.tile([F + 1, G], fp32, tag="gpool")
            nc.vector.memset(gpo[:], 0.0)
            for c in range(NCORES):
                pf = sp.tile([F + 1, 128], fp32, tag="pf")
                nc.sync.dma_start(pf[:], agp_out[c * (F + 1):(c + 1) * (F + 1), :])
                span = int(gspan[c])
                off = int(gbase[c])
                nc.vector.tensor_tensor(
                    out=gpo[:, off:off + span], in0=gpo[:, off:off + span],
                    in1=pf[:, 0:span], op=Alu.add)

            # mean + MLP head
            g1aug = pp.tile([F + 1, G], fp32, tag="g1aug")
            nc.vector.memset(g1aug[F:F + 1, :], 1.0)
            gmean = pp.tile([F, G], fp32, tag="gmean")
            for t in range(G // 128):
                tp = psB.tile([128, F + 1], fp32, tag="tp")
                nc.tensor.transpose(
                    tp[:], gpo[:, t * 128:(t + 1) * 128], ident[0:F + 1, 0:F + 1])
                gpT = sp.tile([128, F + 1], fp32, tag="gpT")
                nc.scalar.activation(gpT[:], tp[:], Act.Copy)
                cnt = sp.tile([128, 1], fp32, tag="cnt")
                nc.vector.tensor_scalar(
                    out=cnt[:], in0=gpT[:, F:F + 1], scalar1=1.0, scalar2=None,
                    op0=Alu.max)
                rec = sp.tile([128, 1], fp32, tag="rec")
                nc.vector.reciprocal(rec[:], cnt[:])
                gmT = sp.tile([128, F], fp32, tag="gmT")
                nc.vector.tensor_scalar(
                    out=gmT[:], in0=gpT[:, 0:F], scalar1=rec[:], scalar2=None,
                    op0=Alu.mult)
                tp2 = psB.tile([128, 128], fp32, tag="tp")
                nc.tensor.transpose(tp2[0:F, 0:128], gmT[:], ident[:])
                nc.scalar.activation(
                    gmean[:, t * 128:(t + 1) * 128], tp2[0:F, 0:128], Act.Copy)

            l1p = psA.tile([F, G], fp32, tag="big")
            nc.tensor.matmul(l1p[:, 0:G], l1w[:], gmean[:], start=True,
                             stop=True)
            nc.scalar.activation(g1aug[0:F, :], l1p[:, 0:G], Act.Relu,
                                 bias=l1b[:])
            for t in range(G // 128):
                zp = psB.tile([128, C], fp32, tag="tp")
                nc.tensor.matmul(
                    zp[:], g1aug[:, t * 128:(t + 1) * 128], l2aug[:],
                    start=True, stop=True)
                z = sp.tile([128, C], fp32, tag="z")
                nc.scalar.activation(z[:], zp[:], Act.Copy)
                m = sp.tile([128, 1], fp32, tag="m")
                nc.vector.reduce_max(m[:], z[:], axis=mybir.AxisListType.X)
                zs = sp.tile([128, C], fp32, tag="zs")
                nc.vector.tensor_scalar(
                    out=zs[:], in0=z[:], scalar1=m[:], scalar2=None,
                    op0=Alu.subtract)
                ex = sp.tile([128, C], fp32, tag="ex")
                se = sp.tile([128, 1], fp32, tag="se")
                nc.scalar.activation(ex[:], zs[:], Act.Exp, accum_out=se[:])
                ls = sp.tile([128, 1], fp32, tag="ls")
                nc.scalar.activation(ls[:], se[:], Act.Ln)
                out_t = sp.tile([128, C], fp32, tag="outt")
                nc.vector.tensor_scalar(
                    out=out_t[:], in0=zs[:], scalar1=ls[:], scalar2=None,
                    op0=Alu.subtract)
                nc.sync.dma_start(y_out[t * 128:(t + 1) * 128, :], out_t[:])

    nc.compile()
    return nc


def kernel(x, edge_index, edge_weight, batch, W1, b1, W2, b2, W3, b3,
           lin1_w, lin1_b, lin2_w, lin2_b):
    from concourse.bass_utils import run_bass_kernel_spmd

    x = np.asarray(x, np.float32)
    meta = _prep(x, edge_index, edge_weight, batch)

    key = "prog"
    if key not in _cache:
        _cache[key] = _build(meta)
    nc = _cache[key]

    x_full = np.zeros((NP, F), np.float32)
    x_full[:N] = x
    iota8 = np.tile(np.arange(128, dtype=np.float32), (128, OHB))
    ident = np.eye(128, dtype=np.float32)
    Wall = np.stack([np.asarray(W1, np.float32), np.asarray(W2, np.float32),
                     np.asarray(W3, np.float32)])  # [3, K, F, H]
    Wall = Wall.reshape(3 * K, F, H).transpose(1, 0, 2).reshape(F, 3 * K * H).copy()
    ball = np.stack([np.asarray(b1, np.float32), np.asarray(b2, np.float32),
                     np.asarray(b3, np.float32)], axis=1)  # [H, 3]
    l2aug = np.concatenate([np.asarray(lin2_w, np.float32),
                            np.asarray(lin2_b, np.float32)[None, :]], axis=0)

    in_maps = []
    for c in range(NCORES):
        xT = x_full[c * SHARD:(c + 1) * SHARD].T.copy()
        in_maps.append({
            "x_full": x_full,
            "xT_shard": xT,
            "idxw": meta["idx_w"][c],
            "ld": meta["ld"][c],
            "nm": meta["nm"][c],
            "gslot": meta["gslot"][c],
            "iota8": iota8,
            "ident": ident,
            "Wall": Wall,
            "ball": ball,
            "l1w": np.asarray(lin1_w, np.float32),
            "l1b": np.asarray(lin1_b, np.float32).reshape(H, 1),
            "l2aug": l2aug,
        })
    res = run_bass_kernel_spmd(nc, in_maps, core_ids=list(range(NCORES)))
    return res.results[0]["y"]


# revision 7
# speedup vs baseline: 1.0022x; 1.0022x over previous
"""ChebNet (K=3, 3 conv layers + MLP head) on 8 Trainium2 NeuronCores.

Strategy: destination-node sharding. Node features h stay replicated in each
core's HBM; each core owns 1/8 of the destination nodes and all edges into
them. A propagate is: dma_gather of h[src] rows (256B each), a per-edge norm
scale on DVE, and a segment-sum via PE matmuls (gathered chunk stationary,
data-dependent one-hot built on DVE as the moving operand), accumulating
feature-major results in PSUM. Shard outputs are exchanged with AllGather
collectives (the graph is random, so halo == everything; full replication of
h is the right call). The small 64x64 weights are replicated; pooling is a
partial segment-sum per shard + one AllGather + local combine.
"""

import numpy as np

N = 50000
E = 800000
F = 64
H = 64
C = 10
G = 512
K = 3
NCORES = 8
NP = 50176          # padded node count: 8 * 6272
SHARD = NP // NCORES  # 6272 = 49 * 128
NBLK = SHARD // 128   # 49 dst blocks of 128 nodes per core
GRP = 4               # blocks per gather group
NGRP = (NBLK + GRP - 1) // GRP  # 13
LO_ROWS = 32768       # int16 gather index limit
HI_ROWS = NP - LO_ROWS
OHB = 8               # one-hot build batch (chunks per DVE op)

_cache = {}


def _prep(x, edge_index, edge_weight, batch):
    """All host-side graph structure preprocessing (numpy)."""
    src = np.asarray(edge_index[0], np.int64)
    dst = np.asarray(edge_index[1], np.int64)
    ew = np.asarray(edge_weight, np.float64)
    w0 = np.where(src == dst, 0.0, ew)
    deg = np.bincount(src, weights=w0, minlength=NP).astype(np.float64)
    dis = np.where(deg > 0, 1.0 / np.sqrt(np.where(deg > 0, deg, 1.0)), 0.0)
    norm = (-dis[src] * w0 * dis[dst]).astype(np.float32)

    core = dst // SHARD
    blk = (dst % SHARD) // 128
    sec = (src >= LO_ROWS).astype(np.int64)  # 0 = lo, 1 = hi

    # order edges by (core, blk, sec) and count
    counts = np.zeros((NCORES, NBLK, 2), np.int64)
    np.add.at(counts, (core, blk, sec), 1)
    budgets = np.maximum(1, np.ceil(counts.max(axis=0) / 128).astype(np.int64))

    # chunk schedule (identical on every core): per group g: lo chunks of its
    # blocks, then hi chunks.
    sched = []  # list of (g, s, b) per chunk, in program order
    ginfo = []  # per (g, s): (chunk_start, nchunks)
    for g in range(NGRP):
        blks = range(g * GRP, min((g + 1) * GRP, NBLK))
        for s in (0, 1):
            start = len(sched)
            for b in blks:
                for _ in range(budgets[b, s]):
                    sched.append((g, s, b))
            ginfo.append((g, s, start, len(sched) - start))
    nch = len(sched)

    # per-core streams
    order = np.lexsort((src, sec, blk, core))  # stable order by core,blk,sec
    so, do, no, co, bo, seco = (a[order] for a in (src, dst, norm, core, blk, sec))
    idx_arr = np.zeros((NCORES, nch * 128), np.int16)
    ld_arr = np.zeros((NCORES, 128, nch), np.float32)
    nm_arr = np.zeros((NCORES, 128, nch), np.float32)

    # chunk offsets per (b, s): start chunk of block b in section s
    chunk_of = {}
    pos = 0
    for g, s, start, n in ginfo:
        blks = list(range(g * GRP, min((g + 1) * GRP, NBLK)))
        cstart = start
        for b in blks:
            chunk_of[(b, s)] = cstart
            cstart += budgets[b, s]

    eptr = np.searchsorted(co, np.arange(NCORES + 1))
    for c in range(NCORES):
        s0, s1 = eptr[c], eptr[c + 1]
        bsec = bo[s0:s1] * 2 + seco[s0:s1]
        bs_ptr = np.searchsorted(bsec, np.arange(2 * NBLK + 1))
        for b in range(NBLK):
            for s in (0, 1):
                lo_, hi_ = bs_ptr[2 * b + s], bs_ptr[2 * b + s + 1]
                cnt = hi_ - lo_
                ck0 = chunk_of[(b, s)]
                sl = slice(s0 + lo_, s0 + hi_)
                e_idx = (so[sl] - (LO_ROWS if s else 0)).astype(np.int16)
                e_ld = (do[sl] % 128).astype(np.float32)
                e_nm = no[sl]
                flat0 = ck0 * 128
                idx_arr[c, flat0:flat0 + cnt] = e_idx
                pp = np.arange(cnt)
                ld_arr[c, pp % 128, ck0 + pp // 128] = e_ld
                nm_arr[c, pp % 128, ck0 + pp // 128] = e_nm

    # wrap idx per gather instruction: [(16, ni//16) -> tile 8x]
    idx_w = np.zeros((NCORES, 128, nch * 8), np.int16)
    for g, s, start, n in ginfo:
        ni = n * 128
        c0 = start * 8
        for c in range(NCORES):
            seg = idx_arr[c, start * 128:start * 128 + ni]
            idx_w[c, :, c0:c0 + ni // 16] = np.tile(
                seg.reshape(ni // 16, 16).T, (8, 1))

    # pooling: graph slots
    batch = np.asarray(batch, np.int64)
    batch_p = np.concatenate([batch, np.full(NP - N, -1, np.int64)])
    gbase = np.zeros(NCORES, np.int64)
    gspan = np.zeros(NCORES, np.int64)
    gslot = np.zeros((NCORES, 128, NBLK), np.float32)
    for c in range(NCORES):
        bseg = batch_p[c * SHARD:(c + 1) * SHARD]
        real = bseg >= 0
        gbase[c] = bseg[real].min()
        gspan[c] = bseg[real].max() - gbase[c] + 1
        slot = np.where(real, bseg - gbase[c], 127).astype(np.float32)
        gslot[c] = slot.reshape(NBLK, 128).T
    assert gspan.max() <= 127

    return dict(budgets=budgets, ginfo=ginfo, sched=sched, nch=nch,
                chunk_of=chunk_of, idx_w=idx_w, ld=ld_arr, nm=nm_arr,
                gslot=gslot, gbase=gbase, gspan=gspan)


def _build(meta):
    import concourse.bacc as bacc
    import concourse.mybir as mybir
    import concourse.tile as tile

    fp32 = mybir.dt.float32
    Alu = mybir.AluOpType
    Act = mybir.ActivationFunctionType
    nch = meta["nch"]
    ginfo = meta["ginfo"]
    budgets = meta["budgets"]
    chunk_of = meta["chunk_of"]
    gbase, gspan = meta["gbase"], meta["gspan"]

    nc = bacc.Bacc("TRN2", target_bir_lowering=False, num_devices=NCORES,
                   num_swdge_queues=4)

    x_dram = nc.dram_tensor("x_full", [NP, F], fp32, kind="ExternalInput")
    xT_in = nc.dram_tensor("xT_shard", [F, SHARD], fp32, kind="ExternalInput")
    idx_in = nc.dram_tensor("idxw", [128, nch * 8], mybir.dt.int16,
                            kind="ExternalInput")
    ld_in = nc.dram_tensor("ld", [128, nch], fp32, kind="ExternalInput")
    nm_in = nc.dram_tensor("nm", [128, nch], fp32, kind="ExternalInput")
    gs_in = nc.dram_tensor("gslot", [128, NBLK], fp32, kind="ExternalInput")
    iota_in = nc.dram_tensor("iota8", [128, OHB * 128], fp32,
                             kind="ExternalInput")
    ident_in = nc.dram_tensor("ident", [128, 128], fp32, kind="ExternalInput")
    w_in = nc.dram_tensor("Wall", [F, 3 * K * H], fp32, kind="ExternalInput")
    b_in = nc.dram_tensor("ball", [H, 3], fp32, kind="ExternalInput")
    l1w_in = nc.dram_tensor("l1w", [H, H], fp32, kind="ExternalInput")
    l1b_in = nc.dram_tensor("l1b", [H, 1], fp32, kind="ExternalInput")
    l2aug_in = nc.dram_tensor("l2aug", [H + 1, C], fp32, kind="ExternalInput")
    y_out = nc.dram_tensor("y", [G, C], fp32, kind="ExternalOutput")

    with tile.TileContext(nc) as tc:
        with (
            tc.tile_pool(name="persist", bufs=1) as pp,
            tc.tile_pool(name="gpool", bufs=2) as gp_,
            tc.tile_pool(name="ohpool", bufs=3) as ohp,
            tc.tile_pool(name="small", bufs=2) as sp,
            tc.tile_pool(name="psA", bufs=2, space="PSUM") as psA,
            tc.tile_pool(name="psB", bufs=2, space="PSUM") as psB,
            tc.tile_pool(name="psP", bufs=4, space="PSUM") as psP,
            tc.tile_pool(name="dram", bufs=1, space="DRAM") as dp,
        ):
            # ---- static loads ----
            idx_sb = pp.tile([128, nch * 8], mybir.dt.int16, tag="idx")
            nc.sync.dma_start(idx_sb[:], idx_in[:])
            ld_sb = pp.tile([128, nch], fp32, tag="ld")
            nc.sync.dma_start(ld_sb[:], ld_in[:])
            nm_sb = pp.tile([128, nch], fp32, tag="nm")
            nc.sync.dma_start(nm_sb[:], nm_in[:])
            gs_sb = pp.tile([128, NBLK], fp32, tag="gs")
            nc.sync.dma_start(gs_sb[:], gs_in[:])
            iota = pp.tile([128, OHB * 128], fp32, tag="iota")
            nc.sync.dma_start(iota[:], iota_in[:])
            ident = pp.tile([128, 128], fp32, tag="ident")
            nc.sync.dma_start(ident[:], ident_in[:])
            w_sb = pp.tile([F, 3 * K * H], fp32, tag="w")
            nc.sync.dma_start(w_sb[:], w_in[:])
            b_sb = pp.tile([H, 3], fp32, tag="b")
            nc.sync.dma_start(b_sb[:], b_in[:])
            l1w = pp.tile([H, H], fp32, tag="l1w")
            nc.sync.dma_start(l1w[:], l1w_in[:])
            l1b = pp.tile([H, 1], fp32, tag="l1b")
            nc.sync.dma_start(l1b[:], l1b_in[:])
            l2aug = pp.tile([H + 1, C], fp32, tag="l2aug")
            nc.sync.dma_start(l2aug[:], l2aug_in[:])

            # feature-major activation tiles [64, SHARD]
            tx0 = pp.tile([F, SHARD], fp32, tag="tx0")
            tx1 = pp.tile([F, SHARD], fp32, tag="tx1")
            tx2 = pp.tile([F, SHARD], fp32, tag="tx2")
            stage = pp.tile([128, NBLK * F], fp32, tag="stage")
            nc.sync.dma_start(tx0[:], xT_in[:])

            # DRAM comm buffers
            agin = [dp.tile([SHARD, F], fp32, tag=f"agin{i}", name=f"agin{i}") for i in range(5)]
            agout = [dp.tile([NP, F], fp32, tag=f"agout{i}", name=f"agout{i}") for i in range(5)]

            def propagate(src_dram, zT):
                """zT[:, :] = feature-major propagate of src_dram rows."""
                for g in range(NGRP):
                    blks = list(range(g * GRP, min((g + 1) * GRP, NBLK)))
                    ncols = len(blks) * 128
                    bt = {b: psP.tile([F, 128], fp32, tag="prop",
                                      name=f"bt{g}_{b}") for b in blks}
                    gt = {}
                    for s in (0, 1):
                        _, _, start, n = ginfo[g * 2 + s]
                        if n == 0:
                            continue
                        gtile = gp_.tile([128, n, F], fp32, tag=f"g{s}")
                        base = src_dram[LO_ROWS:NP, :] if s else \
                            src_dram[0:LO_ROWS, :]
                        nc.gpsimd.dma_gather(
                            out_ap=gtile[:],
                            in_ap=base,
                            idxs_ap=idx_sb[:, start * 8:(start + n) * 8],
                            num_idxs=n * 128,
                            num_idxs_reg=n * 128,
                            elem_size=F,
                            queue_num=(g * 2 + s) % 4,
                            single_packet=False,
                        )
                        # per-edge norm scale (broadcast norm along features)
                        nc.vector.tensor_tensor(
                            out=gtile[:], in0=gtile[:],
                            in1=nm_sb[:, start:start + n].unsqueeze(2)
                            .broadcast_to([128, n, F]),
                            op=Alu.mult)
                        gt[s] = (gtile, start, n)
                    # one-hot builds (batched) + matmuls
                    for s in (0, 1):
                        if s not in gt:
                            continue
                        gtile, start, n = gt[s]
                        for c0 in range(0, n, OHB):
                            nb = min(OHB, n - c0)
                            oh = ohp.tile([128, OHB, 128], fp32, tag="oh")
                            nc.vector.tensor_tensor(
                                out=oh[:, 0:nb, :],
                                in0=iota[:].rearrange(
                                    "p (c f) -> p c f", f=128)[:, 0:nb, :],
                                in1=ld_sb[:, start + c0:start + c0 + nb]
                                .unsqueeze(2).broadcast_to([128, nb, 128]),
                                op=Alu.is_equal)
                            for cc in range(nb):
                                ch = start + c0 + cc
                                _, ss, bb = meta["sched"][ch]
                                bi = bb - blks[0]
                                first = (ch == chunk_of[(bb, 0)])
                                last = (ch == chunk_of[(bb, 1)] +
                                        budgets[bb, 1] - 1)
                                nc.tensor.matmul(
                                    bt[bb][:],
                                    gtile[:, c0 + cc, :],
                                    oh[:, cc, :],
                                    start=first, stop=last)
                    for b in blks:
                        nc.scalar.activation(
                            zT[:, b * 128:(b + 1) * 128], bt[b][:], Act.Copy)

            def transpose_back(zT, out_stage):
                """[64, SHARD] feature-major -> node-major [128, NBLK, 64]."""
                for t in range(NBLK):
                    tp = psB.tile([128, F], fp32, tag="tp")
                    nc.tensor.transpose(
                        tp[:], zT[:, t * 128:(t + 1) * 128], ident[0:F, 0:F])
                    nc.scalar.activation(
                        out_stage[:, t * F:(t + 1) * F], tp[:], Act.Copy)

            def exchange(zT, idx):
                transpose_back(zT, stage)
                nc.sync.dma_start(
                    agin[idx][:].rearrange("(t p) f -> p t f", p=128),
                    stage[:].rearrange("p (t f) -> p t f", f=F))
                nc.gpsimd.collective_compute(
                    "AllGather", mybir.AluOpType.bypass,
                    replica_groups=[list(range(NCORES))],
                    ins=[agin[idx].opt()],
                    outs=[agout[idx].opt()])
                return agout[idx]

            # ---- 3 conv layers ----
            slots = [(tx0, tx1, tx2, tx2), (tx2, tx1, tx0, tx0),
                     (tx0, tx1, tx2, tx2)]
            src = x_dram
            agi = 0
            for L in range(3):
                t0, t1, t2, ho = slots[L]
                propagate(src, t1)
                t1full = exchange(t1, agi)
                agi += 1
                propagate(t1full, t2)
                # t2 = 2*t2 - t0
                nc.vector.tensor_scalar(
                    out=t2[:], in0=t2[:], scalar1=2.0, scalar2=None,
                    op0=Alu.mult)
                nc.vector.tensor_tensor(
                    out=t2[:], in0=t2[:], in1=t0[:], op=Alu.subtract)
                # combo: ho = relu(W0^T t0 + W1^T t1 + W2^T t2 + b)
                for tt in range(NBLK * 128 // 512):
                    cs = tt * 512
                    cp = psA.tile([F, 512], fp32, tag="big")
                    for k, tk in enumerate((t0, t1, t2)):
                        wk = w_sb[:, (L * K + k) * H:(L * K + k + 1) * H]
                        nc.tensor.matmul(
                            cp[:], wk, tk[:, cs:cs + 512],
                            start=(k == 0), stop=(k == 2))
                    nc.scalar.activation(
                        ho[:, cs:cs + 512], cp[:], Act.Relu,
                        bias=b_sb[:, L:L + 1])
                if L < 2:
                    src = exchange(ho, agi)
                    agi += 1

            # ---- pooling (h3 = hout of conv3 = slots[2][3]) ----
            h3 = slots[2][3]
            h3aug = pp.tile([128, NBLK, F + 1], fp32, tag="h3aug")
            nc.vector.memset(h3aug[:, :, F:F + 1], 1.0)
            for t in range(NBLK):
                tp = psB.tile([128, F], fp32, tag="tp")
                nc.tensor.transpose(
                    tp[:], h3[:, t * 128:(t + 1) * 128], ident[0:F, 0:F])
                nc.scalar.activation(h3aug[:, t, 0:F], tp[:], Act.Copy)
            plp = psA.tile([F + 1, 512], fp32, tag="big")
            for t in range(NBLK):
                goh = ohp.tile([128, 128], fp32, tag="goh")
                nc.vector.tensor_scalar(
                    out=goh[:], in0=iota[:, 0:128],
                    scalar1=gs_sb[:, t:t + 1], scalar2=None, op0=Alu.is_equal)
                nc.tensor.matmul(plp[:, 0:128], h3aug[:, t, :], goh[:],
                                 start=(t == 0), stop=(t == NBLK - 1))
            ppart = sp.tile([F + 1, 128], fp32, tag="ppart")
            nc.scalar.activation(ppart[:], plp[:, 0:128], Act.Copy)

            agp_in = dp.tile([F + 1, 128], fp32, tag="agpin")
            agp_out = dp.tile([(F + 1) * NCORES, 128], fp32, tag="agpout")
            nc.sync.dma_start(agp_in[:], ppart[:])
            nc.gpsimd.collective_compute(
                "AllGather", mybir.AluOpType.bypass,
                replica_groups=[list(range(NCORES))],
                ins=[agp_in.opt()], outs=[agp_out.opt()])

            # combine partial pools -> gpool [65, G]
            gpo = pp.tile([F + 1, G], fp32, tag="gpool")
            nc.vector.memset(gpo[:], 0.0)
            for c in range(NCORES):
                pf = sp.tile([F + 1, 128], fp32, tag="pf")
                nc.sync.dma_start(pf[:], agp_out[c * (F + 1):(c + 1) * (F + 1), :])
                span = int(gspan[c])
                off = int(gbase[c])
                nc.vector.tensor_tensor(
                    out=gpo[:, off:off + span], in0=gpo[:, off:off + span],
                    in1=pf[:, 0:span], op=Alu.add)

            # mean + MLP head
            g1aug = pp.tile([F + 1, G], fp32, tag="g1aug")
            nc.vector.memset(g1aug[F:F + 1, :], 1.0)
            gmean = pp.tile([F, G], fp32, tag="gmean")
            for t in range(G // 128):
                tp = psB.tile([128, F + 1], fp32, tag="tp")
                nc.tensor.transpose(
                    tp[:], gpo[:, t * 128:(t + 1) * 128], ident[0:F + 1, 0:F + 1])
                gpT = sp.tile([128, F + 1], fp32, tag="gpT")
                nc.scalar.activation(gpT[:], tp[:], Act.Copy)
                cnt = sp.tile([128, 1], fp32, tag="cnt")
                nc.vector.tensor_scalar(
                    out=cnt[:], in0=gpT[:, F:F + 1], scalar1=1.0, scalar2=None,
                    op0=Alu.max)
                rec = sp.tile([128, 1], fp32, tag="rec")
                nc.vector.reciprocal(rec[:], cnt[:])
                gmT = sp.tile([128, F], fp32, tag="gmT")
                nc.vector.tensor_scalar(
                    out=gmT[:], in0=gpT[:, 0:F], scalar1=rec[:], scalar2=None,
                    op0=Alu.mult)
                tp2 = psB.tile([128, 128], fp32, tag="tp")
                nc.tensor.transpose(tp2[0:F, 0:128], gmT[:], ident[:])
                nc.scalar.activation(
                    gmean[:, t * 128:(t + 1) * 128], tp2[0:F, 0:128], Act.Copy)

            l1p = psA.tile([F, G], fp32, tag="big")
            nc.tensor.matmul(l1p[:, 0:G], l1w[:], gmean[:], start=True,
                             stop=True)
            nc.scalar.activation(g1aug[0:F, :], l1p[:, 0:G], Act.Relu,
                                 bias=l1b[:])
            for t in range(G // 128):
                zp = psB.tile([128, C], fp32, tag="tp")
                nc.tensor.matmul(
                    zp[:], g1aug[:, t * 128:(t + 1) * 128], l2aug[:],
                    start=True, stop=True)
                z = sp.tile([128, C], fp32, tag="z")
                nc.scalar.activation(z[:], zp[:], Act.Copy)
                m = sp.tile([128, 1], fp32, tag="m")
                nc.vector.reduce_max(m[:], z[:], axis=mybir.AxisListType.X)
                zs = sp.tile([128, C], fp32, tag="zs")
                nc.vector.tensor_scalar(
                    out=zs[:], in0=z[:], scalar1=m[:], scalar2=None,
                    op0=Alu.subtract)
                ex = sp.tile([128, C], fp32, tag="ex")
                se = sp.tile([128, 1], fp32, tag="se")
                nc.scalar.activation(ex[:], zs[:], Act.Exp, accum_out=se[:])
                ls = sp.tile([128, 1], fp32, tag="ls")
                nc.scalar.activation(ls[:], se[:], Act.Ln)
                out_t = sp.tile([128, C], fp32, tag="outt")
                nc.vector.tensor_scalar(
                    out=out_t[:], in0=zs[:], scalar1=ls[:], scalar2=None,
                    op0=Alu.subtract)
                nc.sync.dma_start(y_out[t * 128:(t + 1) * 128, :], out_t[:])

    nc.compile()
    return nc


def kernel(x, edge_index, edge_weight, batch, W1, b1, W2, b2, W3, b3,
           lin1_w, lin1_b, lin2_w, lin2_b):
    from concourse.bass_utils import run_bass_kernel_spmd

    x = np.asarray(x, np.float32)
    meta = _prep(x, edge_index, edge_weight, batch)

    key = "prog"
    if key not in _cache:
        _cache[key] = _build(meta)
    nc = _cache[key]

    x_full = np.zeros((NP, F), np.float32)
    x_full[:N] = x
    iota8 = np.tile(np.arange(128, dtype=np.float32), (128, OHB))
    ident = np.eye(128, dtype=np.float32)
    Wall = np.stack([np.asarray(W1, np.float32), np.asarray(W2, np.float32),
                     np.asarray(W3, np.float32)])  # [3, K, F, H]
    Wall = Wall.reshape(3 * K, F, H).transpose(1, 0, 2).reshape(F, 3 * K * H).copy()
    ball = np.stack([np.asarray(b1, np.float32), np.asarray(b2, np.float32),
                     np.asarray(b3, np.float32)], axis=1)  # [H, 3]
    l2aug = np.concatenate([np.asarray(lin2_w, np.float32),
                            np.asarray(lin2_b, np.float32)[None, :]], axis=0)

    in_maps = []
    for c in range(NCORES):
        xT = x_full[c * SHARD:(c + 1) * SHARD].T.copy()
        in_maps.append({
            "x_full": x_full,
            "xT_shard": xT,
            "idxw": meta["idx_w"][c],
            "ld": meta["ld"][c],
            "nm": meta["nm"][c],
            "gslot": meta["gslot"][c],
            "iota8": iota8,
            "ident": ident,
            "Wall": Wall,
            "ball": ball,
            "l1w": np.asarray(lin1_w, np.float32),
            "l1b": np.asarray(lin1_b, np.float32).reshape(H, 1),
            "l2aug": l2aug,
        })
    res = run_bass_kernel_spmd(nc, in_maps, core_ids=list(range(NCORES)))
    return res.results[0]["y"]


# revision 16
# speedup vs baseline: 1.1155x; 1.1130x over previous
"""ChebNet (K=3, 3 conv layers + MLP head) on 8 Trainium2 NeuronCores.

Strategy: destination-node sharding. Node features h stay replicated in each
core's HBM; each core owns 1/8 of the destination nodes and all edges into
them. A propagate is: dma_gather of h[src] rows (256B each), a per-edge norm
scale on DVE, and a segment-sum via PE matmuls (gathered chunk stationary,
data-dependent one-hot built on DVE as the moving operand), accumulating
feature-major results in PSUM. Shard outputs are exchanged with AllGather
collectives (the graph is random, so halo == everything; full replication of
h is the right call). The small 64x64 weights are replicated; pooling is a
partial segment-sum per shard + one AllGather + local combine.
"""

import numpy as np

N = 50000
E = 800000
F = 64
H = 64
C = 10
G = 512
K = 3
NCORES = 8
NP = 50176          # padded node count: 8 * 6272
SHARD = NP // NCORES  # 6272 = 49 * 128
NBLK = SHARD // 128   # 49 dst blocks of 128 nodes per core
GRP = 4               # blocks per gather group
NGRP = (NBLK + GRP - 1) // GRP  # 13
LO_ROWS = 32768       # int16 gather index limit
HI_ROWS = NP - LO_ROWS
OHB = 8               # one-hot build batch (chunks per DVE op)

_cache = {}


def _prep(x, edge_index, edge_weight, batch):
    """All host-side graph structure preprocessing (numpy)."""
    src = np.asarray(edge_index[0], np.int64)
    dst = np.asarray(edge_index[1], np.int64)
    ew = np.asarray(edge_weight, np.float64)
    w0 = np.where(src == dst, 0.0, ew)
    deg = np.bincount(src, weights=w0, minlength=NP).astype(np.float64)
    dis = np.where(deg > 0, 1.0 / np.sqrt(np.where(deg > 0, deg, 1.0)), 0.0)
    norm = (-dis[src] * w0 * dis[dst]).astype(np.float32)

    core = dst // SHARD
    blk = (dst % SHARD) // 128
    sec = (src >= LO_ROWS).astype(np.int64)  # 0 = lo, 1 = hi

    # order edges by (core, blk, sec) and count
    counts = np.zeros((NCORES, NBLK, 2), np.int64)
    np.add.at(counts, (core, blk, sec), 1)
    budgets = np.maximum(1, np.ceil(counts.max(axis=0) / 128).astype(np.int64))

    # chunk schedule (identical on every core): per group g: lo chunks of its
    # blocks, then hi chunks.
    sched = []  # list of (g, s, b) per chunk, in program order
    ginfo = []  # per (g, s): (chunk_start, nchunks)
    for g in range(NGRP):
        blks = range(g * GRP, min((g + 1) * GRP, NBLK))
        for s in (0, 1):
            start = len(sched)
            for b in blks:
                for _ in range(budgets[b, s]):
                    sched.append((g, s, b))
            ginfo.append((g, s, start, len(sched) - start))
    nch = len(sched)

    # per-core streams
    order = np.lexsort((src, sec, blk, core))  # stable order by core,blk,sec
    so, do, no, co, bo, seco = (a[order] for a in (src, dst, norm, core, blk, sec))
    idx_arr = np.zeros((NCORES, nch * 128), np.int16)
    ld_arr = np.zeros((NCORES, 128, nch), np.float32)
    nm_arr = np.zeros((NCORES, 128, nch), np.float32)

    # chunk offsets per (b, s): start chunk of block b in section s
    chunk_of = {}
    pos = 0
    for g, s, start, n in ginfo:
        blks = list(range(g * GRP, min((g + 1) * GRP, NBLK)))
        cstart = start
        for b in blks:
            chunk_of[(b, s)] = cstart
            cstart += budgets[b, s]

    eptr = np.searchsorted(co, np.arange(NCORES + 1))
    for c in range(NCORES):
        s0, s1 = eptr[c], eptr[c + 1]
        bsec = bo[s0:s1] * 2 + seco[s0:s1]
        bs_ptr = np.searchsorted(bsec, np.arange(2 * NBLK + 1))
        for b in range(NBLK):
            for s in (0, 1):
                lo_, hi_ = bs_ptr[2 * b + s], bs_ptr[2 * b + s + 1]
                cnt = hi_ - lo_
                ck0 = chunk_of[(b, s)]
                sl = slice(s0 + lo_, s0 + hi_)
                e_idx = (so[sl] - (LO_ROWS if s else 0)).astype(np.int16)
                e_ld = (do[sl] % 128).astype(np.float32)
                e_nm = no[sl]
                flat0 = ck0 * 128
                idx_arr[c, flat0:flat0 + cnt] = e_idx
                pp = np.arange(cnt)
                ld_arr[c, pp % 128, ck0 + pp // 128] = e_ld
                nm_arr[c, pp % 128, ck0 + pp // 128] = e_nm

    # wrap idx per gather instruction: [(16, ni//16) -> tile 8x]
    idx_w = np.zeros((NCORES, 128, nch * 8), np.int16)
    for g, s, start, n in ginfo:
        ni = n * 128
        c0 = start * 8
        for c in range(NCORES):
            seg = idx_arr[c, start * 128:start * 128 + ni]
            idx_w[c, :, c0:c0 + ni // 16] = np.tile(
                seg.reshape(ni // 16, 16).T, (8, 1))

    # pooling: graph slots
    batch = np.asarray(batch, np.int64)
    batch_p = np.concatenate([batch, np.full(NP - N, -1, np.int64)])
    gbase = np.zeros(NCORES, np.int64)
    gspan = np.zeros(NCORES, np.int64)
    gslot = np.zeros((NCORES, 128, NBLK), np.float32)
    for c in range(NCORES):
        bseg = batch_p[c * SHARD:(c + 1) * SHARD]
        real = bseg >= 0
        gbase[c] = bseg[real].min()
        gspan[c] = bseg[real].max() - gbase[c] + 1
        slot = np.where(real, bseg - gbase[c], 127).astype(np.float32)
        gslot[c] = slot.reshape(NBLK, 128).T
    assert gspan.max() <= 127

    return dict(budgets=budgets, ginfo=ginfo, sched=sched, nch=nch,
                chunk_of=chunk_of, idx_w=idx_w, ld=ld_arr, nm=nm_arr,
                gslot=gslot, gbase=gbase, gspan=gspan)


def _build(meta):
    import concourse.bacc as bacc
    import concourse.mybir as mybir
    import concourse.tile as tile

    fp32 = mybir.dt.float32
    Alu = mybir.AluOpType
    Act = mybir.ActivationFunctionType
    nch = meta["nch"]
    ginfo = meta["ginfo"]
    budgets = meta["budgets"]
    chunk_of = meta["chunk_of"]
    gbase, gspan = meta["gbase"], meta["gspan"]

    nc = bacc.Bacc("TRN2", target_bir_lowering=False, num_devices=NCORES,
                   num_swdge_queues=4)

    x_dram = nc.dram_tensor("x_full", [NP, F], fp32, kind="ExternalInput")
    xT_in = nc.dram_tensor("xT_shard", [F, SHARD], fp32, kind="ExternalInput")
    idx_in = nc.dram_tensor("idxw", [128, nch * 8], mybir.dt.int16,
                            kind="ExternalInput")
    ld_in = nc.dram_tensor("ld", [128, nch], fp32, kind="ExternalInput")
    nm_in = nc.dram_tensor("nm", [128, nch], fp32, kind="ExternalInput")
    gs_in = nc.dram_tensor("gslot", [128, NBLK], fp32, kind="ExternalInput")
    iota_in = nc.dram_tensor("iota8", [128, OHB * 128], fp32,
                             kind="ExternalInput")
    ident_in = nc.dram_tensor("ident", [128, 128], fp32, kind="ExternalInput")
    w_in = nc.dram_tensor("Wall", [F, 3 * K * H], fp32, kind="ExternalInput")
    b_in = nc.dram_tensor("ball", [H, 3], fp32, kind="ExternalInput")
    l1w_in = nc.dram_tensor("l1w", [H, H], fp32, kind="ExternalInput")
    l1b_in = nc.dram_tensor("l1b", [H, 1], fp32, kind="ExternalInput")
    l2aug_in = nc.dram_tensor("l2aug", [H + 1, C], fp32, kind="ExternalInput")
    y_out = nc.dram_tensor("y", [G, C], fp32, kind="ExternalOutput")

    with tile.TileContext(nc) as tc:
        with (
            tc.tile_pool(name="persist", bufs=1) as pp,
            tc.tile_pool(name="gpool", bufs=2) as gp_,
            tc.tile_pool(name="ohpool", bufs=3) as ohp,
            tc.tile_pool(name="small", bufs=2) as sp,
            tc.tile_pool(name="psA", bufs=2, space="PSUM") as psA,
            tc.tile_pool(name="psB", bufs=2, space="PSUM") as psB,
            tc.tile_pool(name="psP", bufs=4, space="PSUM") as psP,
            tc.tile_pool(name="dram", bufs=1, space="DRAM") as dp,
        ):
            # ---- static loads ----
            idx_sb = pp.tile([128, nch * 8], mybir.dt.int16, tag="idx")
            nc.sync.dma_start(idx_sb[:], idx_in[:])
            ld_sb = pp.tile([128, nch], fp32, tag="ld")
            nc.sync.dma_start(ld_sb[:], ld_in[:])
            nm_sb = pp.tile([128, nch], fp32, tag="nm")
            nc.sync.dma_start(nm_sb[:], nm_in[:])
            gs_sb = pp.tile([128, NBLK], fp32, tag="gs")
            nc.sync.dma_start(gs_sb[:], gs_in[:])
            iota = pp.tile([128, OHB * 128], fp32, tag="iota")
            nc.sync.dma_start(iota[:], iota_in[:])
            ident = pp.tile([128, 128], fp32, tag="ident")
            nc.sync.dma_start(ident[:], ident_in[:])
            w_sb = pp.tile([F, 3 * K * H], fp32, tag="w")
            nc.sync.dma_start(w_sb[:], w_in[:])
            b_sb = pp.tile([H, 3], fp32, tag="b")
            nc.sync.dma_start(b_sb[:], b_in[:])
            l1w = pp.tile([H, H], fp32, tag="l1w")
            nc.sync.dma_start(l1w[:], l1w_in[:])
            l1b = pp.tile([H, 1], fp32, tag="l1b")
            nc.sync.dma_start(l1b[:], l1b_in[:])
            l2aug = pp.tile([H + 1, C], fp32, tag="l2aug")
            nc.sync.dma_start(l2aug[:], l2aug_in[:])

            # feature-major activation tiles [64, SHARD]
            tx0 = pp.tile([F, SHARD], fp32, tag="tx0")
            tx1 = pp.tile([F, SHARD], fp32, tag="tx1")
            tx2 = pp.tile([F, SHARD], fp32, tag="tx2")
            stage = pp.tile([128, NBLK * F], fp32, tag="stage")
            nc.sync.dma_start(tx0[:], xT_in[:])

            # DRAM comm buffers (real tensors: dma_gather needs fixed addrs)
            agin = [nc.dram_tensor(f"agin{i}", [SHARD, F], fp32)
                    for i in range(5)]
            agout = [nc.dram_tensor(f"agout{i}", [NP, F], fp32)
                     for i in range(5)]

            def propagate(src_dram, zT):
                """zT[:, :] = feature-major propagate of src_dram rows."""
                for g in range(NGRP):
                    blks = list(range(g * GRP, min((g + 1) * GRP, NBLK)))
                    ncols = len(blks) * 128
                    bt = {b: psP.tile([F, 128], fp32, tag="prop",
                                      name=f"bt{g}_{b}") for b in blks}
                    gt = {}
                    for s in (0, 1):
                        _, _, start, n = ginfo[g * 2 + s]
                        if n == 0:
                            continue
                        gtile = gp_.tile([128, n, F], fp32, tag=f"g{s}")
                        base = src_dram[LO_ROWS:NP, :] if s else \
                            src_dram[0:LO_ROWS, :]
                        nc.gpsimd.dma_gather(
                            out_ap=gtile[:],
                            in_ap=base,
                            idxs_ap=idx_sb[:, start * 8:(start + n) * 8],
                            num_idxs=n * 128,
                            num_idxs_reg=n * 128,
                            elem_size=F,
                            queue_num=(g * 2 + s) % 4,
                            single_packet=False,
                        )
                        # per-edge norm scale (broadcast norm along features)
                        nc.vector.tensor_tensor(
                            out=gtile[:], in0=gtile[:],
                            in1=nm_sb[:, start:start + n].unsqueeze(2)
                            .broadcast_to([128, n, F]),
                            op=Alu.mult)
                        gt[s] = (gtile, start, n)
                    # one-hot builds (batched) + matmuls
                    for s in (0, 1):
                        if s not in gt:
                            continue
                        gtile, start, n = gt[s]
                        for c0 in range(0, n, OHB):
                            nb = min(OHB, n - c0)
                            oh = ohp.tile([128, OHB, 128], fp32, tag="oh")
                            nc.vector.tensor_tensor(
                                out=oh[:, 0:nb, :],
                                in0=iota[:].rearrange(
                                    "p (c f) -> p c f", f=128)[:, 0:nb, :],
                                in1=ld_sb[:, start + c0:start + c0 + nb]
                                .unsqueeze(2).broadcast_to([128, nb, 128]),
                                op=Alu.is_equal)
                            for cc in range(nb):
                                ch = start + c0 + cc
                                _, ss, bb = meta["sched"][ch]
                                bi = bb - blks[0]
                                first = (ch == chunk_of[(bb, 0)])
                                last = (ch == chunk_of[(bb, 1)] +
                                        budgets[bb, 1] - 1)
                                nc.tensor.matmul(
                                    bt[bb][:],
                                    gtile[:, c0 + cc, :],
                                    oh[:, cc, :],
                                    start=first, stop=last)
                    for b in blks:
                        nc.scalar.activation(
                            zT[:, b * 128:(b + 1) * 128], bt[b][:], Act.Copy)

            def transpose_back(zT, out_stage):
                """[64, SHARD] feature-major -> node-major [128, NBLK, 64]."""
                for t in range(NBLK):
                    tp = psB.tile([128, F], fp32, tag="tp")
                    nc.tensor.transpose(
                        tp[:], zT[:, t * 128:(t + 1) * 128], ident[0:F, 0:F])
                    nc.scalar.activation(
                        out_stage[:, t * F:(t + 1) * F], tp[:], Act.Copy)

            def exchange(zT, idx):
                transpose_back(zT, stage)
                nc.sync.dma_start(
                    agin[idx].ap().rearrange("(t p) f -> p t f", p=128),
                    stage[:].rearrange("p (t f) -> p t f", f=F))
                nc.gpsimd.collective_compute(
                    "AllGather", mybir.AluOpType.bypass,
                    replica_groups=[list(range(NCORES))],
                    ins=[agin[idx].ap().opt()],
                    outs=[agout[idx].ap().opt()])
                # custom dma_gather needs a linearly-mapped (IO) source;
                # x_dram is dead after conv1's first propagate, so stage all
                # exchanged full-h buffers through it.
                for cc in range(NCORES):
                    nc.sync.dma_start(
                        stage[:].rearrange("p (t f) -> p t f", f=F),
                        agout[idx][cc * SHARD:(cc + 1) * SHARD, :]
                        .rearrange("(t p) f -> p t f", p=128))
                    nc.sync.dma_start(
                        x_dram[cc * SHARD:(cc + 1) * SHARD, :]
                        .rearrange("(t p) f -> p t f", p=128),
                        stage[:].rearrange("p (t f) -> p t f", f=F))
                return x_dram

            # ---- 3 conv layers ----
            slots = [(tx0, tx1, tx2, tx2), (tx2, tx1, tx0, tx0),
                     (tx0, tx1, tx2, tx2)]
            src = x_dram
            agi = 0
            for L in range(3):
                t0, t1, t2, ho = slots[L]
                propagate(src, t1)
                t1full = exchange(t1, agi)
                agi += 1
                propagate(t1full, t2)
                # t2 = 2*t2 - t0
                nc.vector.tensor_scalar(
                    out=t2[:], in0=t2[:], scalar1=2.0, scalar2=None,
                    op0=Alu.mult)
                nc.vector.tensor_tensor(
                    out=t2[:], in0=t2[:], in1=t0[:], op=Alu.subtract)
                # combo: ho = relu(W0^T t0 + W1^T t1 + W2^T t2 + b)
                for cs in range(0, SHARD, 512):
                    cw = min(512, SHARD - cs)
                    cp = psA.tile([F, 512], fp32, tag="big")
                    for k, tk in enumerate((t0, t1, t2)):
                        wk = w_sb[:, (L * K + k) * H:(L * K + k + 1) * H]
                        nc.tensor.matmul(
                            cp[:, 0:cw], wk, tk[:, cs:cs + cw],
                            start=(k == 0), stop=(k == 2))
                    nc.scalar.activation(
                        ho[:, cs:cs + cw], cp[:, 0:cw], Act.Relu,
                        bias=b_sb[:, L:L + 1])
                if L < 2:
                    src = exchange(ho, agi)
                    agi += 1

            # ---- pooling (h3 = hout of conv3 = slots[2][3]) ----
            h3 = slots[2][3]
            h3aug = pp.tile([128, NBLK, F + 1], fp32, tag="h3aug")
            nc.vector.memset(h3aug[:, :, F:F + 1], 1.0)
            for t in range(NBLK):
                tp = psB.tile([128, F], fp32, tag="tp")
                nc.tensor.transpose(
                    tp[:], h3[:, t * 128:(t + 1) * 128], ident[0:F, 0:F])
                nc.scalar.activation(h3aug[:, t, 0:F], tp[:], Act.Copy)
            plp = psA.tile([F + 1, 512], fp32, tag="big")
            for t in range(NBLK):
                goh = ohp.tile([128, 128], fp32, tag="goh")
                nc.vector.tensor_scalar(
                    out=goh[:], in0=iota[:, 0:128],
                    scalar1=gs_sb[:, t:t + 1], scalar2=None, op0=Alu.is_equal)
                nc.tensor.matmul(plp[:, 0:128], h3aug[:, t, :], goh[:],
                                 start=(t == 0), stop=(t == NBLK - 1))
            ppart = sp.tile([F + 1, 128], fp32, tag="ppart")
            nc.scalar.activation(ppart[:], plp[:, 0:128], Act.Copy)

            agp_in = nc.dram_tensor("agp_in", [F + 1, 128], fp32)
            agp_out = nc.dram_tensor("agp_out", [(F + 1) * NCORES, 128], fp32)
            nc.sync.dma_start(agp_in[:], ppart[:])
            nc.gpsimd.collective_compute(
                "AllGather", mybir.AluOpType.bypass,
                replica_groups=[list(range(NCORES))],
                ins=[agp_in.ap().opt()], outs=[agp_out.ap().opt()])

            # combine partial pools -> gpool [65, G]
            gpo = pp.tile([F + 1, G], fp32, tag="gpool")
            nc.vector.memset(gpo[:], 0.0)
            for c in range(NCORES):
                pf = sp.tile([F + 1, 128], fp32, tag="pf")
                nc.sync.dma_start(pf[:], agp_out[c * (F + 1):(c + 1) * (F + 1), :])
                span = int(gspan[c])
                off = int(gbase[c])
                nc.vector.tensor_tensor(
                    out=gpo[:, off:off + span], in0=gpo[:, off:off + span],
                    in1=pf[:, 0:span], op=Alu.add)

            # mean + MLP head
            g1aug = pp.tile([F + 1, G], fp32, tag="g1aug")
            nc.vector.memset(g1aug[F:F + 1, :], 1.0)
            gmean = pp.tile([F, G], fp32, tag="gmean")
            for t in range(G // 128):
                tp = psB.tile([128, F + 1], fp32, tag="tp")
                nc.tensor.transpose(
                    tp[:], gpo[:, t * 128:(t + 1) * 128], ident[0:F + 1, 0:F + 1])
                gpT = sp.tile([128, F + 1], fp32, tag="gpT")
                nc.scalar.activation(gpT[:], tp[:], Act.Copy)
                cnt = sp.tile([128, 1], fp32, tag="cnt")
                nc.vector.tensor_scalar(
                    out=cnt[:], in0=gpT[:, F:F + 1], scalar1=1.0, scalar2=None,
                    op0=Alu.max)
                rec = sp.tile([128, 1], fp32, tag="rec")
                nc.vector.reciprocal(rec[:], cnt[:])
                gmT = sp.tile([128, F], fp32, tag="gmT")
                nc.vector.tensor_scalar(
                    out=gmT[:], in0=gpT[:, 0:F], scalar1=rec[:], scalar2=None,
                    op0=Alu.mult)
                tp2 = psB.tile([128, 128], fp32, tag="tp")
                nc.tensor.transpose(tp2[0:F, 0:128], gmT[:], ident[:])
                nc.scalar.activation(
                    gmean[:, t * 128:(t + 1) * 128], tp2[0:F, 0:128], Act.Copy)

            l1p = psA.tile([F, G], fp32, tag="big")
            nc.tensor.matmul(l1p[:, 0:G], l1w[:], gmean[:], start=True,
                             stop=True)
            nc.scalar.activation(g1aug[0:F, :], l1p[:, 0:G], Act.Relu,
                                 bias=l1b[:])
            for t in range(G // 128):
                zp = psB.tile([128, C], fp32, tag="tp")
                nc.tensor.matmul(
                    zp[:], g1aug[:, t * 128:(t + 1) * 128], l2aug[:],
                    start=True, stop=True)
                z = sp.tile([128, C], fp32, tag="z")
                nc.scalar.activation(z[:], zp[:], Act.Copy)
                m = sp.tile([128, 1], fp32, tag="m")
                nc.vector.reduce_max(m[:], z[:], axis=mybir.AxisListType.X)
                zs = sp.tile([128, C], fp32, tag="zs")
                nc.vector.tensor_scalar(
                    out=zs[:], in0=z[:], scalar1=m[:], scalar2=None,
                    op0=Alu.subtract)
                ex = sp.tile([128, C], fp32, tag="ex")
                se = sp.tile([128, 1], fp32, tag="se")
                nc.scalar.activation(ex[:], zs[:], Act.Exp, accum_out=se[:])
                ls = sp.tile([128, 1], fp32, tag="ls")
                nc.scalar.activation(ls[:], se[:], Act.Ln)
                out_t = sp.tile([128, C], fp32, tag="outt")
                nc.vector.tensor_scalar(
                    out=out_t[:], in0=zs[:], scalar1=ls[:], scalar2=None,
                    op0=Alu.subtract)
                nc.sync.dma_start(y_out[t * 128:(t + 1) * 128, :], out_t[:])

    nc.compile()
    return nc


def kernel(x, edge_index, edge_weight, batch, W1, b1, W2, b2, W3, b3,
           lin1_w, lin1_b, lin2_w, lin2_b):
    from concourse.bass_utils import run_bass_kernel_spmd

    x = np.asarray(x, np.float32)
    meta = _prep(x, edge_index, edge_weight, batch)

    key = "prog"
    if key not in _cache:
        _cache[key] = _build(meta)
    nc = _cache[key]

    x_full = np.zeros((NP, F), np.float32)
    x_full[:N] = x
    iota8 = np.tile(np.arange(128, dtype=np.float32), (128, OHB))
    ident = np.eye(128, dtype=np.float32)
    Wall = np.stack([np.asarray(W1, np.float32), np.asarray(W2, np.float32),
                     np.asarray(W3, np.float32)])  # [3, K, F, H]
    Wall = Wall.reshape(3 * K, F, H).transpose(1, 0, 2).reshape(F, 3 * K * H).copy()
    ball = np.stack([np.asarray(b1, np.float32), np.asarray(b2, np.float32),
                     np.asarray(b3, np.float32)], axis=1)  # [H, 3]
    l2aug = np.concatenate([np.asarray(lin2_w, np.float32),
                            np.asarray(lin2_b, np.float32)[None, :]], axis=0)

    in_maps = []
    for c in range(NCORES):
        xT = x_full[c * SHARD:(c + 1) * SHARD].T.copy()
        in_maps.append({
            "x_full": x_full,
            "xT_shard": xT,
            "idxw": meta["idx_w"][c],
            "ld": meta["ld"][c],
            "nm": meta["nm"][c],
            "gslot": meta["gslot"][c],
            "iota8": iota8,
            "ident": ident,
            "Wall": Wall,
            "ball": ball,
            "l1w": np.asarray(lin1_w, np.float32),
            "l1b": np.asarray(lin1_b, np.float32).reshape(H, 1),
            "l2aug": l2aug,
        })
    res = run_bass_kernel_spmd(nc, in_maps, core_ids=list(range(NCORES)))
    return res.results[0]["y"]
